# revision 28
# baseline (speedup 1.0000x reference)
"""Trainium2 Bass kernel for nn_AttentionBlock (GroupNorm + 8-head self-attention).

Data-parallel over batch: 8 batch elements -> 8 NeuronCores, one each.

Per-core layout ([c, n] with c on partitions, c = 4 chunks x 128, n = 1024):
  GroupNorm   : bn_stats per channel -> group-combine matmul (G) ->
                broadcast matmul (G^T) -> fused scale/shift -> xn (bf16)
  QKV         : Q,K in [o, n] layout (lhsT = wqkvT chunks, bf16),
                V in [n, o] layout (V^T, lhsT = xn chunks); emitted interleaved
                with attention pairs so PE fills exp-wait gaps
  Attention   : per head-pair p (heads 2p, 2p+1), per m-chunk s:
                  S^T[m,:] = K^T Q   (bf16 N=1024, 2 heads row-packed)
                  P = exp(S/8)       (ScalarE, PSUM->SBUF [128,1024], bf16)
                  rowsums via ones-vector matmuls (M=1, rows 0/64, packed)
                  att_un += V^T.T @ P (2 heads col-packed, M=64)
                  normalize: att = att_un * (1/rowsum) via approx-recip +
                  DRAM-roundtrip broadcast DMA
  Proj        : fp32r matmul + residual add, output fp32
"""

import numpy as np

NUM_GROUPS = 32
NUM_HEADS = 8
EPS = 1e-6
C = 512
N = 1024
B = 8

_cache = {}


def _build_bass():
    import concourse.bacc as bacc
    import concourse.bass as bass_mod
    import concourse.mybir as mybir
    import concourse.tile as tile

    fp32 = mybir.dt.float32
    fp32r = mybir.dt.float32r
    bf16 = mybir.dt.bfloat16
    AF = mybir.ActivationFunctionType
    OP = mybir.AluOpType

    nc = bacc.Bacc("TRN2", target_bir_lowering=False, debug=False)

    x_d = nc.dram_tensor("x", [C, N], fp32, kind="ExternalInput")
    wqkvT_d = nc.dram_tensor("wqkvT", [C, 3 * C], fp32, kind="ExternalInput")
    projT_d = nc.dram_tensor("projT", [C, C], fp32, kind="ExternalInput")
    qkvb_d = nc.dram_tensor("qkv_b", [3 * C], fp32, kind="ExternalInput")
    projb_d = nc.dram_tensor("proj_b", [C], fp32, kind="ExternalInput")
    nw_d = nc.dram_tensor("norm_w", [C], fp32, kind="ExternalInput")
    nb_d = nc.dram_tensor("norm_b", [C], fp32, kind="ExternalInput")
    G_d = nc.dram_tensor("Gmat", [128, 4, 32], fp32, kind="ExternalInput")
    GT_d = nc.dram_tensor("GTmat", [32, 4, 128], fp32, kind="ExternalInput")
    y_d = nc.dram_tensor("y", [C, N], fp32, kind="ExternalOutput")
    r8r_d = nc.dram_tensor("r8r_scratch", [4, 4, 512], fp32)

    with tile.TileContext(nc) as tc:
        with (
            tc.tile_pool(name="const", bufs=1) as const,
            tc.tile_pool(name="work", bufs=1) as work,
            tc.tile_pool(name="ppool", bufs=2) as ppool,
            tc.tile_pool(name="rot", bufs=2) as rot,
            tc.tile_pool(name="pss", bufs=2, space="PSUM") as pss,     # 2x[128,1024]
            tc.tile_pool(name="psav", bufs=1, space="PSUM") as psav,   # [128,1024]
            tc.tile_pool(name="psr", bufs=1, space="PSUM") as psr,
            tc.tile_pool(name="psq", bufs=1, space="PSUM") as psq,     # [128,1024]
        ):
            # ---------------- load inputs ----------------
            x_sb = work.tile([128, 4, N], fp32, tag="x")
            _dma_engines = [nc.sync, nc.scalar, nc.gpsimd, nc.sync]
            for j in range(4):
                _dma_engines[j].dma_start(
                    x_sb[:, j, :], x_d.ap().rearrange("(j p) n -> j p n", p=128)[j]
                )

            w_bf = work.tile([128, 4, 3 * C], bf16, tag="wbf")
            p_r = work.tile([128, 4, C], fp32r, tag="pr")
            for j in range(4):
                stg = rot.tile([128, 3 * C], fp32, tag="stage")
                [nc.scalar, nc.sync, nc.scalar, nc.sync][j].dma_start(
                    stg[:], wqkvT_d.ap().rearrange("(j p) o -> j p o", p=128)[j]
                )
                nc.vector.tensor_copy(w_bf[:, j, :], stg[:])
            for j in range(4):
                stg = rot.tile([128, C], fp32, tag="stage2")
                nc.sync.dma_start(
                    stg[:], projT_d.ap().rearrange("(j p) o -> j p o", p=128)[j]
                )
                nc.scalar.copy(p_r[:, j, :], stg[:])

            G_sb = const.tile([128, 4, 32], fp32, tag="G")
            GT_sb = const.tile([32, 4, 128], fp32, tag="GT")
            nc.sync.dma_start(G_sb[:], G_d.ap())
            nc.sync.dma_start(GT_sb[:], GT_d.ap())
            nw_sb = const.tile([128, 4], fp32, tag="nw")
            nb_sb = const.tile([128, 4], fp32, tag="nb")
            nc.sync.dma_start(nw_sb[:], nw_d.ap().rearrange("(j p) -> p j", p=128))
            nc.sync.dma_start(nb_sb[:], nb_d.ap().rearrange("(j p) -> p j", p=128))
            qb_sb = const.tile([128, 8], fp32, tag="qb")
            nc.sync.dma_start(
                qb_sb[:], qkvb_d.ap()[0 : 2 * C].rearrange("(o p) -> p o", p=128)
            )
            pb_sb = const.tile([128, 4], fp32, tag="pb")
            nc.sync.dma_start(pb_sb[:], projb_d.ap().rearrange("(j p) -> p j", p=128))
            vb_src = qkvb_d.ap()[2 * C : 3 * C]
            vb_bcast_ap = bass_mod.AP(
                tensor=vb_src.tensor, offset=vb_src.offset, ap=[[0, 128], [1, C]]
            )
            vb_bc = const.tile([128, C], fp32, tag="vbbc")
            nc.sync.dma_start(vb_bc[:], vb_bcast_ap)
            ones_bf = const.tile([128, 1], bf16, tag="ones")
            nc.vector.memset(ones_bf[:], 1.0)
            # pre-warm the exp/ln ACT table set during input DMA
            warm = const.tile([32, 1], fp32, tag="warm")
            nc.vector.memset(warm[:], 1.0)
            nc.scalar.activation(warm[:], warm[:], AF.Exp, scale=1.0)

            # ---------------- groupnorm ----------------
            stats = work.tile([128, 4, 2, 6], fp32, tag="stats")
            for j in range(4):
                for u in range(2):
                    nc.vector.bn_stats(
                        stats[:, j, u, :], x_sb[:, j, u * 512 : u * 512 + 512]
                    )
            mv = work.tile([128, 4, 2], fp32, tag="mv")
            for j in range(4):
                nc.vector.bn_aggr(mv[:, j, :], stats[:, j, :, :])
            ssq = work.tile([128, 4, 2], fp32, tag="ssq")
            nc.vector.tensor_copy(ssq[:, :, 0], mv[:, :, 0])
            nc.vector.tensor_tensor(ssq[:, :, 1], mv[:, :, 0], mv[:, :, 0], op=OP.mult)
            nc.vector.tensor_tensor(ssq[:, :, 1], ssq[:, :, 1], mv[:, :, 1], op=OP.add)
            # group stats [32, 2] = (mu_g, E[x^2]_g); G has 1/16 entries
            ps_g = psr.tile([32, 2], fp32, tag="r")
            for j in range(4):
                nc.tensor.matmul(
                    ps_g[:], G_sb[:, j, :], ssq[:, j, :], start=(j == 0), stop=(j == 3)
                )
            st2 = work.tile([32, 2], fp32, tag="st2")
            nc.vector.tensor_copy(st2[:, 0:1], ps_g[:, 0:1])
            var = work.tile([32, 1], fp32, tag="var")
            nc.vector.tensor_tensor(var[:], st2[:, 0:1], st2[:, 0:1], op=OP.mult)
            nc.vector.tensor_tensor(var[:], ps_g[:, 1:2], var[:], op=OP.subtract)
            eps_sb = const.tile([32, 1], fp32, tag="eps")
            nc.vector.memset(eps_sb[:], float(EPS))
            nc.scalar.activation(var[:], var[:], AF.Ln, bias=eps_sb[:], scale=1.0)
            nc.scalar.activation(st2[:, 1:2], var[:], AF.Exp, scale=-0.5)
            ps_bc = psr.tile([128, 4, 2], fp32, tag="r")
            for j in range(4):
                nc.tensor.matmul(
                    ps_bc[:, j, :], GT_sb[:, j, :], st2[:], start=True, stop=True
                )
            ab = work.tile([128, 4, 2], fp32, tag="ab")
            nc.vector.tensor_tensor(ab[:, :, 0], ps_bc[:, :, 1], nw_sb[:], op=OP.mult)
            nc.vector.tensor_tensor(ab[:, :, 1], ps_bc[:, :, 0], ab[:, :, 0], op=OP.mult)
            nc.vector.tensor_tensor(ab[:, :, 1], nb_sb[:], ab[:, :, 1], op=OP.subtract)

            xn_bf = work.tile([128, 4, N], bf16, tag="xn")
            for j in range(4):
                nc.vector.tensor_scalar(
                    xn_bf[:, j, :],
                    x_sb[:, j, :],
                    ab[:, j, 0:1],
                    ab[:, j, 1:2],
                    op0=OP.mult,
                    op1=OP.add,
                )

            # ---------------- V^T = xn^T @ wV^T : [n, o] ----------------
            VT_bf = work.tile([128, 8, C], bf16, tag="VT")

            def emit_v_chunks(mcs):
                for mc in mcs:
                    ps_v = psq.tile([128, 512], fp32, tag="q", name="ps_v")
                    for k in range(4):
                        nc.tensor.matmul(
                            ps_v[:],
                            xn_bf[:, k, mc * 128 : mc * 128 + 128],
                            w_bf[:, k, 2 * C : 3 * C],
                            start=(k == 0),
                            stop=(k == 3),
                        )
                    nc.vector.tensor_tensor(
                        VT_bf[:, mc, :], ps_v[:], vb_bc[:], op=OP.add
                    )

            # ---------------- Q, K chunks (emitted interleaved) ----------------
            QK_bf = work.tile([128, 8, N], bf16, tag="QK")

            def emit_qk(oc):
                ps_qk = pss.tile([128, 1024], fp32, tag="s")
                for nu in range(2):
                    for k in range(4):
                        nc.tensor.matmul(
                            ps_qk[:, nu * 512 : nu * 512 + 512],
                            w_bf[:, k, oc * 128 : oc * 128 + 128],
                            xn_bf[:, k, nu * 512 : nu * 512 + 512],
                            start=(k == 0),
                            stop=(k == 3),
                        )
                nc.vector.tensor_scalar(
                    QK_bf[:, oc, :], ps_qk[:], qb_sb[:, oc : oc + 1], None, op0=OP.add
                )

            def emit_qk_small(oc):
                # mid-attention QK: one psum bank, two half generations
                for nu in range(2):
                    ps_qk = psq.tile([128, 512], fp32, tag="q", name="ps_qk")
                    for k in range(4):
                        nc.tensor.matmul(
                            ps_qk[:],
                            w_bf[:, k, oc * 128 : oc * 128 + 128],
                            xn_bf[:, k, nu * 512 : nu * 512 + 512],
                            start=(k == 0),
                            stop=(k == 3),
                        )
                    nc.vector.tensor_scalar(
                        QK_bf[:, oc, nu * 512 : nu * 512 + 512],
                        ps_qk[:],
                        qb_sb[:, oc : oc + 1],
                        None,
                        op0=OP.add,
                    )

            rr = work.tile([128, 4, 512], fp32, tag="rr")
            R_sb = work.tile([128, 4, N], fp32, tag="R")
            att = work.tile([128, 4, N], fp32, tag="att")

            def emit_pair(p, fillers={}):
                P_bf = ppool.tile([128, 8, 2, 2, 512], bf16, tag="P")
                ps_av = psav.tile([128, 1024], fp32, tag="av")
                ps_r = psr.tile([128, 512], fp32, tag="r")
                nc.vector.memset(ps_r[:], 1.0)

                def s_unit(s, nu):
                    # S^T for both heads: A -> [:, 0:512], B -> [:, 512:1024]
                    u = pss.tile([128, 1024], fp32, tag="s", name="ps_s")
                    for e in range(2):
                        nc.tensor.matmul(
                            u[:, e * 512 : e * 512 + 512],
                            QK_bf[e * 64 : e * 64 + 64, 4 + p, s * 128 : s * 128 + 128],
                            QK_bf[e * 64 : e * 64 + 64, p, nu * 512 : nu * 512 + 512],
                            start=True,
                            stop=True,
                            skip_group_check=True,
                        )
                    return u

                def exp_unit(s, nu, u):
                    nc.scalar.activation(
                        P_bf[:, s, nu, :, :],
                        u[:].rearrange("q (h n) -> q h n", h=2),
                        AF.Exp,
                        scale=0.125,
                    )

                def ones_unit(s, nu):
                    for e in range(2):
                        row = e * 64 + nu * 32
                        nc.tensor.matmul(
                            ps_r[row : row + 1, :],
                            ones_bf[:],
                            P_bf[:, s, nu, e, :],
                            start=(s == 0),
                            stop=(s == 7),
                            tile_position=(0, row),
                            skip_group_check=True,
                        )

                def av_unit(s, nu):
                    for e in range(2):
                        nc.tensor.matmul(
                            ps_av[e * 64 : e * 64 + 64, nu * 512 : nu * 512 + 512],
                            VT_bf[:, s, p * 128 + e * 64 : p * 128 + e * 64 + 64],
                            P_bf[:, s, nu, e, :],
                            start=(s == 0),
                            stop=(s == 7),
                            skip_group_check=True,
                        )

                units = [None, None]
                units[0] = s_unit(0, 0)
                units[1] = s_unit(0, 1)
                for s in range(8):
                    for nu in range(2):
                        exp_unit(s, nu, units[nu])
                    if s < 7:
                        units[0] = s_unit(s + 1, 0)
                        units[1] = s_unit(s + 1, 1)
                    for nu in range(2):
                        ones_unit(s, nu)
                    for nu in range(2):
                        av_unit(s, nu)
                    if s in fillers:
                        fillers[s]()
                # reciprocal of rowsums (~4e-6 rel); rows 0/32/64/96 meaningful
                nc.vector.reciprocal_approx_fast(rr[:, p, :], ps_r[:])
                # roundtrip broadcast: rows (e, nu) -> R_sb
                for e in range(2):
                    for nu in range(2):
                        row = e * 64 + nu * 32
                        nc.sync.dma_start(
                            r8r_d.ap()[p, 2 * e + nu][None, :],
                            rr[row : row + 1, p, :],
                        )
                for e in range(2):
                    src_ap = r8r_d.ap()[p, 2 * e]
                    bcast = bass_mod.AP(
                        tensor=src_ap.tensor,
                        offset=src_ap.offset,
                        ap=[[0, 64], [512, 2], [1, 512]],
                    )
                    nc.sync.dma_start(
                        R_sb[e * 64 : e * 64 + 64, p, :].rearrange(
                            "q (u n) -> q u n", u=2
                        ),
                        bcast,
                    )
                # drain attnV accumulator (rounds to fp32r for the proj matmul)
                nc.vector.tensor_copy(att.bitcast(fp32r)[:, p, :], ps_av[:])
                nc.vector.tensor_tensor(
                    att.bitcast(fp32r)[:, p, :], att[:, p, :], R_sb[:, p, :], op=OP.mult
                )

            emit_qk(0)
            emit_qk(4)
            emit_v_chunks([0, 1])
            emit_pair(0, {
                0: lambda: emit_v_chunks([2, 3, 4, 5]),
                2: lambda: (emit_v_chunks([6, 7]), emit_qk_small(1)),
                5: lambda: emit_qk_small(5),
            })
            emit_pair(1, {2: lambda: emit_qk_small(2), 5: lambda: emit_qk_small(6)})
            emit_pair(2, {2: lambda: emit_qk_small(3), 5: lambda: emit_qk_small(7)})
            emit_pair(3)

            # ---------------- proj + residual ----------------
            att_r = att.bitcast(fp32r)
            for j in range(4):
                nc.vector.tensor_scalar(
                    x_sb[:, j, :], x_sb[:, j, :], pb_sb[:, j : j + 1], None, op0=OP.add
                )
            for oc in range(4):
                pool_ = pss if oc % 2 == 0 else psav
                ps_o = pool_.tile([128, 1024], fp32, tag="s" if oc % 2 == 0 else "av")
                for nu in range(2):
                    for k in range(4):
                        nc.tensor.matmul(
                            ps_o[:, nu * 512 : nu * 512 + 512],
                            p_r[:, k, oc * 128 : oc * 128 + 128],
                            att_r[:, k, nu * 512 : nu * 512 + 512],
                            start=(k == 0),
                            stop=(k == 3),
                        )
                nc.vector.tensor_tensor(
                    x_sb[:, oc, :], ps_o[:], x_sb[:, oc, :], op=OP.add
                )
                _dma_engines[oc].dma_start(
                    y_d.ap().rearrange("(j p) n -> j p n", p=128)[oc], x_sb[:, oc, :]
                )

    nc.compile()
    return nc


def _get_nc(debug=False):
    if "nc" not in _cache:
        _cache["nc"] = _build_bass()
    return _cache["nc"]


def _host_inputs(x, norm_w, norm_b, qkv_w, qkv_b, proj_w, proj_b):
    x = np.asarray(x, dtype=np.float32).reshape(B, C, N)
    wqkvT = np.ascontiguousarray(np.asarray(qkv_w, dtype=np.float32).T)
    projT = np.ascontiguousarray(np.asarray(proj_w, dtype=np.float32).T)
    G = np.zeros((128, 4, 32), dtype=np.float32)
    GT = np.zeros((32, 4, 128), dtype=np.float32)
    for j in range(4):
        for p in range(128):
            g = 8 * j + p // 16
            G[p, j, g] = 1.0 / 16.0
            GT[g, j, p] = 1.0
    shared = {
        "wqkvT": wqkvT,
        "projT": projT,
        "qkv_b": np.asarray(qkv_b, dtype=np.float32),
        "proj_b": np.asarray(proj_b, dtype=np.float32),
        "norm_w": np.asarray(norm_w, dtype=np.float32),
        "norm_b": np.asarray(norm_b, dtype=np.float32),
        "Gmat": G,
        "GTmat": GT,
    }
    in_maps = [dict(shared, x=np.ascontiguousarray(x[i])) for i in range(B)]
    return in_maps


def kernel(x, norm_w, norm_b, qkv_w, qkv_b, proj_w, proj_b, _trace=False):
    from concourse import bass_utils

    nc = _get_nc()
    in_maps = _host_inputs(x, norm_w, norm_b, qkv_w, qkv_b, proj_w, proj_b)
    res = bass_utils.run_bass_kernel_spmd(
        nc, in_maps, core_ids=list(range(B)), trace=_trace
    )
    out = np.stack([res.results[i]["y"] for i in range(B)])
    _cache["last_result"] = res
    return out.reshape(B, C, 32, 32)
